# revision 4
# baseline (speedup 1.0000x reference)
"""GRU kernel for Trainium2, 8 NeuronCores, data-parallel over batch.

Problem: B=64, S=1024, I=H=1024 GRU (bias_ih only).
  gi = x @ W_ih.T + b_ih            (big parallel GEMM)
  per step: gh = h @ W_hh.T; gates; h' = (1-z)*n + z*h   (serial, S steps)

Sharding: batch 64 -> 8 per core; weights replicated. All matmul operands
bf16 (fp32 matmul is 4x slower on PE; bf16 numerics validated at ~3e-3 rel).

Layouts (per core, BL=8 local batch):
  Everything "transposed": hidden dim on partitions, batch in free dim.
  hT  [128, k*8+b]  = h[b, k*128+p]           (k = H-chunk 0..7)
  ghT psum [128, m*8+b] for m-tile m (0..23 over 3H)
  gi DRAM [128, t*192 + m*8 + b]  (step slices contiguous [128,192])
  x host-pre-transposed to [ic, 128, S, BL] bf16 so GEMM moving operand
  DMAs are contiguous. y written back in native [128, t*64+k*8+b] layout,
  un-transposed on host.
"""

import os
import sys

import numpy as np
import ml_dtypes

for _p in ("/opt/trn_rl_repo",):
    if _p not in sys.path:
        sys.path.insert(0, _p)

import concourse.bass as bass
import concourse.bacc as bacc
import concourse.mybir as mybir
from concourse import tile
from concourse.bass_utils import run_bass_kernel_spmd

B, S, I, H = 64, 1024, 1024, 1024
NC = 8
BL = B // NC            # 8
G3 = 3 * H              # 3072
MT = G3 // 128          # 24 m-tiles over 3H
KT = H // 128           # 8 k-chunks over H
ICT = I // 128          # 8 i-chunks over I
TCG = 64                # gemm chunk (steps)
TCR = 32                # recurrence chunk (steps)
F32 = mybir.dt.float32
BF16 = mybir.dt.bfloat16
SIG = mybir.ActivationFunctionType.Sigmoid
TANH = mybir.ActivationFunctionType.Tanh


def build():
    nc = bacc.Bacc("TRN2", target_bir_lowering=False, debug=False)
    xT = nc.declare_dram_parameter("xT", [ICT, 128, S, BL], BF16, isOutput=False)
    h0T = nc.declare_dram_parameter("h0T", [128, KT * BL], F32, isOutput=False)
    WihT = nc.declare_dram_parameter("WihT", [ICT, 128, G3], BF16, isOutput=False)
    WhhT = nc.declare_dram_parameter("WhhT", [KT, 128, G3], BF16, isOutput=False)
    bih = nc.declare_dram_parameter("bih", [128, MT], F32, isOutput=False)
    y_raw = nc.declare_dram_parameter("y_raw", [128, S * KT * BL], F32, isOutput=True)
    hx_raw = nc.declare_dram_parameter("hx_raw", [128, KT * BL], F32, isOutput=True)
    gi_dram = nc.dram_tensor("gi_buf", [128, S * MT * BL], F32)

    with tile.TileContext(nc) as tc:
        # ---------------- phase 1: gi = x @ W_ih.T + b_ih -> DRAM ----------
        with (
            tc.tile_pool(name="w1", bufs=1) as wpool,
            tc.tile_pool(name="p1", bufs=2, space="PSUM") as ppool,
            tc.tile_pool(name="s1", bufs=2) as spool,
        ):
            wih_sb = wpool.tile([128, ICT * G3], BF16)
            for ic in range(ICT):
                nc.sync.dma_start(wih_sb[:, ic * G3 : (ic + 1) * G3], WihT[ic])
            b_sb = wpool.tile([128, MT], F32)
            nc.sync.dma_start(b_sb[:], bih[:])

            N1 = TCG * BL  # 512 moving columns per chunk
            with tc.For_i(0, S // TCG, 1) as ci:
                xt_sb = spool.tile([128, ICT, N1], BF16, tag="xt")
                for ic in range(ICT):
                    nc.sync.dma_start(
                        xt_sb[:, ic, :],
                        xT[ic][:, bass.ds(ci * TCG, TCG), :],
                    )
                stage = spool.tile([128, TCG, MT * BL], F32, tag="gistage")
                for m in range(MT):
                    ps = ppool.tile([128, TCG, BL], F32, tag="ps1")
                    for k in range(ICT):
                        nc.tensor.matmul(
                            ps[:],
                            wih_sb[:, k * G3 + m * 128 : k * G3 + (m + 1) * 128],
                            xt_sb[:, k, :],
                            start=(k == 0),
                            stop=(k == ICT - 1),
                        )
                    # scatter into per-step layout, fused + bias
                    nc.vector.tensor_scalar_add(
                        stage[:, :, m * BL : (m + 1) * BL],
                        ps[:],
                        b_sb[:, m : m + 1],
                    )
                nc.sync.dma_start(
                    gi_dram[:, bass.ds(ci * (TCG * MT * BL), TCG * MT * BL)],
                    stage[:],
                )

        # ---------------- phase 2: recurrence ------------------------------
        with (
            tc.tile_pool(name="w2", bufs=1) as wpool2,
            tc.tile_pool(name="pers", bufs=1) as pers,
            tc.tile_pool(name="p2", bufs=2, space="PSUM") as ppool2,
            tc.tile_pool(name="s2", bufs=2) as spool2,
            tc.tile_pool(name="tmp", bufs=2) as tpool,
        ):
            whh_sb = wpool2.tile([128, KT * G3], BF16)
            for k in range(KT):
                nc.sync.dma_start(whh_sb[:, k * G3 : (k + 1) * G3], WhhT[k])

            hTf = pers.tile([128, KT * BL], F32)
            hTb = pers.tile([128, KT * BL], BF16)
            nc.sync.dma_start(hTf[:], h0T[:])
            nc.vector.tensor_copy(hTb[:], hTf[:])

            NG = MT * BL            # 192 gate columns per step
            N2 = TCR * NG           # gi chunk columns
            NY = KT * BL            # 64

            with tc.For_i(0, S // TCR, 1) as ci:
                gi_sb = spool2.tile([128, N2], F32, tag="gi")
                nc.sync.dma_start(gi_sb[:], gi_dram[:, bass.ds(ci * N2, N2)])
                y_stage = spool2.tile([128, TCR * NY], F32, tag="y")

                for t in range(TCR):
                    gis = gi_sb[:, t * NG : (t + 1) * NG]
                    ps_rz = ppool2.tile([128, 128], F32, tag="psrz")
                    ps_n = ppool2.tile([128, NY], F32, tag="psn")
                    for m in range(MT):
                        out = (
                            ps_rz[:, m * BL : (m + 1) * BL]
                            if m < 16
                            else ps_n[:, (m - 16) * BL : (m - 15) * BL]
                        )
                        for k in range(KT):
                            nc.tensor.matmul(
                                out,
                                whh_sb[:, k * G3 + m * 128 : k * G3 + (m + 1) * 128],
                                hTb[:, k * BL : (k + 1) * BL],
                                start=(k == 0),
                                stop=(k == KT - 1),
                            )
                    s_rz = tpool.tile([128, 128], F32, tag="srz")
                    nc.vector.tensor_add(s_rz[:], ps_rz[:], gis[:, 0:128])
                    rz = tpool.tile([128, 128], F32, tag="rz")
                    nc.scalar.activation(rz[:], s_rz[:], SIG)
                    u = tpool.tile([128, NY], F32, tag="u")
                    nc.vector.tensor_mul(u[:], rz[:, 0:NY], ps_n[:])
                    v = tpool.tile([128, NY], F32, tag="v")
                    nc.vector.tensor_add(v[:], u[:], gis[:, 128:192])
                    nst = tpool.tile([128, NY], F32, tag="nst")
                    nc.scalar.activation(nst[:], v[:], TANH)
                    w = tpool.tile([128, NY], F32, tag="w")
                    nc.vector.tensor_sub(w[:], hTf[:], nst[:])
                    q = tpool.tile([128, NY], F32, tag="q")
                    nc.vector.tensor_mul(q[:], rz[:, NY:128], w[:])
                    nc.vector.tensor_add(hTf[:], nst[:], q[:])
                    nc.vector.tensor_copy(y_stage[:, t * NY : (t + 1) * NY], hTf[:])
                    nc.vector.tensor_copy(hTb[:], hTf[:])

                nc.sync.dma_start(
                    y_raw[:, bass.ds(ci * (TCR * NY), TCR * NY)], y_stage[:]
                )
            nc.sync.dma_start(hx_raw[:], hTf[:])
    nc.compile()
    return nc


_CACHE = {}


def _prep_inputs(x, h0, W_ih, W_hh, b_ih):
    bf = ml_dtypes.bfloat16
    x = np.ascontiguousarray(np.asarray(x, dtype=np.float32))
    h0 = np.asarray(h0, dtype=np.float32)
    W_ih = np.asarray(W_ih, dtype=np.float32)
    W_hh = np.asarray(W_hh, dtype=np.float32)
    b_ih = np.asarray(b_ih, dtype=np.float32)

    # shared weights
    wihT = np.ascontiguousarray(W_ih.T).reshape(ICT, 128, G3).astype(bf)
    whhT = np.ascontiguousarray(W_hh.T).reshape(KT, 128, G3).astype(bf)
    b_arr = np.ascontiguousarray(b_ih.reshape(MT, 128).T)

    in_maps = []
    for c in range(NC):
        x_c = x[c * BL : (c + 1) * BL]                      # [BL, S, I]
        xT_c = np.ascontiguousarray(x_c.transpose(2, 1, 0)) # [I, S, BL]
        xT_c = xT_c.reshape(ICT, 128, S, BL).astype(bf)
        h0_c = h0[c * BL : (c + 1) * BL]                    # [BL, H]
        h0T_c = np.ascontiguousarray(
            h0_c.reshape(BL, KT, 128).transpose(2, 1, 0).reshape(128, KT * BL)
        )
        in_maps.append(
            {"xT": xT_c, "h0T": h0T_c, "WihT": wihT, "WhhT": whhT, "bih": b_arr}
        )
    return in_maps


def _postprocess(results):
    y_full = np.empty((B, S, H), dtype=np.float32)
    hx_full = np.empty((B, H), dtype=np.float32)
    for c in range(NC):
        y_raw = results[c]["y_raw"]        # [128, S*KT*BL]
        hx_raw = results[c]["hx_raw"]      # [128, KT*BL]
        y = y_raw.reshape(128, S, KT, BL).transpose(3, 1, 2, 0).reshape(BL, S, H)
        y_full[c * BL : (c + 1) * BL] = y
        hx = hx_raw.reshape(128, KT, BL).transpose(2, 1, 0).reshape(BL, H)
        hx_full[c * BL : (c + 1) * BL] = hx
    return y_full, hx_full


def kernel(x, h0, W_ih, W_hh, b_ih):
    if "nc" not in _CACHE:
        _CACHE["nc"] = build()
    nc = _CACHE["nc"]
    in_maps = _prep_inputs(x, h0, W_ih, W_hh, b_ih)
    trace = bool(int(os.environ.get("GRU_TRACE", "0")))
    res = run_bass_kernel_spmd(nc, in_maps, list(range(NC)), trace=trace)
    _CACHE["last_result"] = res
    return _postprocess(res.results)


if __name__ == "__main__":
    rng = np.random.default_rng(0)
    sc = 1.0 / np.sqrt(H)
    inputs = {
        "x": rng.standard_normal((B, S, I), dtype=np.float32),
        "h0": np.zeros((B, H), dtype=np.float32),
        "W_ih": (rng.standard_normal((G3, I), dtype=np.float32) * sc),
        "W_hh": (rng.standard_normal((G3, H), dtype=np.float32) * sc),
        "b_ih": (rng.standard_normal(G3, dtype=np.float32) * sc),
    }
    y, hx = kernel(**inputs)
    print("ok", y.shape, hx.shape, float(np.abs(y).max()))


# revision 6
# speedup vs baseline: 1.3503x; 1.3503x over previous
"""GRU kernel for Trainium2, 8 NeuronCores, data-parallel over batch.

Problem: B=64, S=1024, I=H=1024 GRU (bias_ih only).
  gi = x @ W_ih.T + b_ih            (big parallel GEMM)
  per step: gh = h @ W_hh.T; gates; h' = (1-z)*n + z*h   (serial, S steps)

Sharding: batch 64 -> 8 per core; weights replicated. All matmul operands
bf16 (fp32 matmul is 4x slower on PE; bf16 numerics ~3e-3 rel-l2).

Layouts (per core, BL=8 local batch): hidden dim on partitions, batch in
free dim ("transposed"), so the serial recurrence needs no on-chip
transposes and gate math uses all 128 partitions.
  hT  [128, k*8+b]  = h[b, k*128+p]           (k = H-chunk 0..7)
  ghT psum [128, m*8+b] for m-tile m (0..23 over 3H)
  gi DRAM [128, t*192 + m*8 + b]  (step slices contiguous [128,192])
x is host-pre-transposed to [ic, 128, S, BL] bf16; y is emitted in the
native [128, t*64+k*8+b] layout and un-transposed on host.
"""

import os
import sys

import numpy as np
import ml_dtypes

for _p in ("/opt/trn_rl_repo",):
    if _p not in sys.path:
        sys.path.insert(0, _p)

import concourse.bass as bass
import concourse.bacc as bacc
import concourse.mybir as mybir
import concourse.bass_utils as _bu
from concourse import tile
from concourse.bass_utils import run_bass_kernel_spmd

B, S, I, H = 64, 1024, 1024, 1024
NC = 8
BL = B // NC            # 8
G3 = 3 * H              # 3072
MT = G3 // 128          # 24 m-tiles over 3H
KT = H // 128           # 8 k-chunks over H
ICT = I // 128          # 8 i-chunks over I
TCG = 64                # gemm chunk (steps)
TCR = 64                # recurrence chunk (steps)
F32 = mybir.dt.float32
BF16 = mybir.dt.bfloat16
SIG = mybir.ActivationFunctionType.Sigmoid
TANH = mybir.ActivationFunctionType.Tanh

# Walrus's LDWEIGHTS optimization (fast weight load) is disabled by the
# default driver flags; the recurrence is LDW-bound so it matters here.
if int(os.environ.get("GRU_LDWOPT", "0")):
    _orig_run_command = _bu.run_command

    def _run_command_ldwopt(argv, **kw):
        argv = [
            "--enable-ldw-opt=true" if a == "--enable-ldw-opt=false" else a
            for a in argv
        ]
        return _orig_run_command(argv, **kw)

    _bu.run_command = _run_command_ldwopt


def build():
    nc = bacc.Bacc("TRN2", target_bir_lowering=False, debug=False)
    xT = nc.declare_dram_parameter("xT", [ICT, 128, S, BL], BF16, isOutput=False)
    h0T = nc.declare_dram_parameter("h0T", [128, KT * BL], F32, isOutput=False)
    WihT = nc.declare_dram_parameter("WihT", [ICT, 128, G3], BF16, isOutput=False)
    WhhT = nc.declare_dram_parameter("WhhT", [KT, 128, G3], BF16, isOutput=False)
    bih = nc.declare_dram_parameter("bih", [128, MT], F32, isOutput=False)
    y_raw = nc.declare_dram_parameter("y_raw", [128, S * KT * BL], F32, isOutput=True)
    hx_raw = nc.declare_dram_parameter("hx_raw", [128, KT * BL], F32, isOutput=True)
    gi_dram = nc.dram_tensor("gi_buf", [128, S * MT * BL], F32)

    with tile.TileContext(nc) as tc:
        # ---------------- phase 1: gi = x @ W_ih.T + b_ih -> DRAM ----------
        with (
            tc.tile_pool(name="w1", bufs=1) as wpool,
            tc.tile_pool(name="p1", bufs=4, space="PSUM") as ppool,
            tc.tile_pool(name="s1", bufs=2) as spool,
        ):
            wih_sb = wpool.tile([128, ICT * G3], BF16)
            for ic in range(ICT):
                nc.sync.dma_start(wih_sb[:, ic * G3 : (ic + 1) * G3], WihT[ic])
            b_sb = wpool.tile([128, MT], F32)
            nc.sync.dma_start(b_sb[:], bih[:])

            N1 = TCG * BL  # 512 moving columns per chunk
            for ci in range(S // TCG):
                xt_sb = spool.tile([128, ICT, N1], BF16, tag="xt")
                for ic in range(ICT):
                    nc.sync.dma_start(
                        xt_sb[:, ic, :],
                        xT[ic][:, ci * TCG : (ci + 1) * TCG, :],
                    )
                stage = spool.tile([128, TCG, MT * BL], F32, tag="gistage")
                for m in range(MT):
                    ps = ppool.tile([128, TCG, BL], F32, tag="ps1")
                    for k in range(ICT):
                        nc.tensor.matmul(
                            ps[:],
                            wih_sb[:, k * G3 + m * 128 : k * G3 + (m + 1) * 128],
                            xt_sb[:, k, :],
                            start=(k == 0),
                            stop=(k == ICT - 1),
                        )
                    # scatter into per-step layout, fused + bias
                    nc.vector.tensor_scalar_add(
                        stage[:, :, m * BL : (m + 1) * BL],
                        ps[:],
                        b_sb[:, m : m + 1],
                    )
                nc.sync.dma_start(
                    gi_dram[:, ci * (TCG * MT * BL) : (ci + 1) * (TCG * MT * BL)],
                    stage[:],
                )

        # ---------------- phase 2: recurrence ------------------------------
        with (
            tc.tile_pool(name="w2", bufs=1) as wpool2,
            tc.tile_pool(name="pers", bufs=1) as pers,
            tc.tile_pool(name="p2", bufs=2, space="PSUM") as ppool2,
            tc.tile_pool(name="s2", bufs=2) as spool2,
            tc.tile_pool(name="tmp", bufs=2) as tpool,
        ):
            whh_sb = wpool2.tile([128, KT * G3], BF16)
            for k in range(KT):
                nc.sync.dma_start(whh_sb[:, k * G3 : (k + 1) * G3], WhhT[k])

            hTf = pers.tile([128, KT * BL], F32)
            hTb = pers.tile([128, KT * BL], BF16)
            nc.sync.dma_start(hTf[:], h0T[:])
            nc.vector.tensor_copy(hTb[:], hTf[:])

            NG = MT * BL            # 192 gate columns per step
            N2 = TCR * NG           # gi chunk columns
            NY = KT * BL            # 64

            with tc.For_i(0, S // TCR, 1) as ci:
                gi_sb = spool2.tile([128, N2], F32, tag="gi")
                # quarter-DMAs so early steps only wait on the first slice
                q = N2 // 4
                for j in range(4):
                    nc.sync.dma_start(
                        gi_sb[:, j * q : (j + 1) * q],
                        gi_dram[:, bass.ds(ci * N2 + j * q, q)],
                    )
                y_stage = spool2.tile([128, TCR * NY], F32, tag="y")

                for t in range(TCR):
                    gis = gi_sb[:, t * NG : (t + 1) * NG]
                    h_prev = hTf[:] if t == 0 else y_stage[:, (t - 1) * NY : t * NY]
                    ps_rz = ppool2.tile([128, 128], F32, tag="psrz")
                    ps_n = ppool2.tile([128, NY], F32, tag="psn")
                    for m in range(MT):
                        out = (
                            ps_rz[:, m * BL : (m + 1) * BL]
                            if m < 16
                            else ps_n[:, (m - 16) * BL : (m - 15) * BL]
                        )
                        for k in range(KT):
                            nc.tensor.matmul(
                                out,
                                whh_sb[:, k * G3 + m * 128 : k * G3 + (m + 1) * 128],
                                hTb[:, k * BL : (k + 1) * BL],
                                start=(k == 0),
                                stop=(k == KT - 1),
                            )
                    # --- overlaps the n-part matmuls ---
                    s_rz = tpool.tile([128, 128], F32, tag="srz")
                    nc.vector.tensor_add(s_rz[:], ps_rz[:], gis[:, 0:128])
                    rz = tpool.tile([128, 128], F32, tag="rz")
                    nc.scalar.activation(rz[:], s_rz[:], SIG)
                    zh = tpool.tile([128, NY], F32, tag="zh")
                    nc.vector.tensor_mul(zh[:], rz[:, NY:128], h_prev)
                    omz = tpool.tile([128, NY], F32, tag="omz")
                    nc.vector.tensor_scalar(
                        omz[:], rz[:, NY:128], -1.0, 1.0,
                        mybir.AluOpType.mult, mybir.AluOpType.add,
                    )
                    # --- critical tail after ps_n ---
                    u = tpool.tile([128, NY], F32, tag="u")
                    nc.vector.tensor_mul(u[:], rz[:, 0:NY], ps_n[:])
                    v = tpool.tile([128, NY], F32, tag="v")
                    nc.vector.tensor_add(v[:], u[:], gis[:, 128:192])
                    nst = tpool.tile([128, NY], F32, tag="nst")
                    nc.scalar.activation(nst[:], v[:], TANH)
                    h1 = tpool.tile([128, NY], F32, tag="h1")
                    nc.vector.tensor_mul(h1[:], nst[:], omz[:])
                    # bf16 h for next step's matmuls: output-cast on the add
                    nc.vector.tensor_add(hTb[:], h1[:], zh[:])
                    # fp32 h (= y_t) off the critical path
                    nc.vector.tensor_add(
                        y_stage[:, t * NY : (t + 1) * NY], h1[:], zh[:]
                    )

                nc.vector.tensor_copy(hTf[:], y_stage[:, (TCR - 1) * NY :])
                nc.sync.dma_start(
                    y_raw[:, bass.ds(ci * (TCR * NY), TCR * NY)], y_stage[:]
                )
            nc.sync.dma_start(hx_raw[:], hTf[:])
    nc.compile()
    return nc


_CACHE = {}


def _prep_inputs(x, h0, W_ih, W_hh, b_ih):
    bf = ml_dtypes.bfloat16
    x = np.ascontiguousarray(np.asarray(x, dtype=np.float32))
    h0 = np.asarray(h0, dtype=np.float32)
    W_ih = np.asarray(W_ih, dtype=np.float32)
    W_hh = np.asarray(W_hh, dtype=np.float32)
    b_ih = np.asarray(b_ih, dtype=np.float32)

    # shared weights
    wihT = np.ascontiguousarray(W_ih.T).reshape(ICT, 128, G3).astype(bf)
    whhT = np.ascontiguousarray(W_hh.T).reshape(KT, 128, G3).astype(bf)
    b_arr = np.ascontiguousarray(b_ih.reshape(MT, 128).T)

    in_maps = []
    for c in range(NC):
        x_c = x[c * BL : (c + 1) * BL]                      # [BL, S, I]
        xT_c = np.ascontiguousarray(x_c.transpose(2, 1, 0)) # [I, S, BL]
        xT_c = xT_c.reshape(ICT, 128, S, BL).astype(bf)
        h0_c = h0[c * BL : (c + 1) * BL]                    # [BL, H]
        h0T_c = np.ascontiguousarray(
            h0_c.reshape(BL, KT, 128).transpose(2, 1, 0).reshape(128, KT * BL)
        )
        in_maps.append(
            {"xT": xT_c, "h0T": h0T_c, "WihT": wihT, "WhhT": whhT, "bih": b_arr}
        )
    return in_maps


def _postprocess(results):
    y_full = np.empty((B, S, H), dtype=np.float32)
    hx_full = np.empty((B, H), dtype=np.float32)
    for c in range(NC):
        y_raw = results[c]["y_raw"]        # [128, S*KT*BL]
        hx_raw = results[c]["hx_raw"]      # [128, KT*BL]
        y = y_raw.reshape(128, S, KT, BL).transpose(3, 1, 2, 0).reshape(BL, S, H)
        y_full[c * BL : (c + 1) * BL] = y
        hx = hx_raw.reshape(128, KT, BL).transpose(2, 1, 0).reshape(BL, H)
        hx_full[c * BL : (c + 1) * BL] = hx
    return y_full, hx_full


def kernel(x, h0, W_ih, W_hh, b_ih):
    if "nc" not in _CACHE:
        _CACHE["nc"] = build()
    nc = _CACHE["nc"]
    in_maps = _prep_inputs(x, h0, W_ih, W_hh, b_ih)
    trace = bool(int(os.environ.get("GRU_TRACE", "0")))
    res = run_bass_kernel_spmd(nc, in_maps, list(range(NC)), trace=trace)
    _CACHE["last_result"] = res
    return _postprocess(res.results)


if __name__ == "__main__":
    rng = np.random.default_rng(0)
    sc = 1.0 / np.sqrt(H)
    inputs = {
        "x": rng.standard_normal((B, S, I), dtype=np.float32),
        "h0": np.zeros((B, H), dtype=np.float32),
        "W_ih": (rng.standard_normal((G3, I), dtype=np.float32) * sc),
        "W_hh": (rng.standard_normal((G3, H), dtype=np.float32) * sc),
        "b_ih": (rng.standard_normal(G3, dtype=np.float32) * sc),
    }
    y, hx = kernel(**inputs)
    print("ok", y.shape, hx.shape, float(np.abs(y).max()))
